# revision 19
# baseline (speedup 1.0000x reference)
"""Trainium2 Bass kernel for DisentangledTransformer sliding-window attention.

Problem: B=16, N=512, D=512, H=8 heads (d=64), relative-position (DeBERTa-style
c2p/p2c) attention with sliding window |i-j| <= 8 and shared q/k projections for
the position embeddings.

Sharding: data-parallel over batch. Each of the 8 cores processes 2 batches with
replicated weights. No collectives.

Per-core algorithm (per batch b, head h):
  - Inputs q,k,v are loaded naturally, PE-transposed to [c, n] layout.
  - Projections produce qlT = (q@Wq.T+bq).T/scale, klT (padded along keys),
    and vl in 5 overlapping "halo" key tiles [128, 512] (+ ones columns for the
    softmax denominator trick).
  - Only 17 diagonals matter (window 8).  Scores are computed per 112-query
    block against its 128-key window:
      c2c block  : PE matmul qlT-slice x klT-window                [112, 128]
      c2p band   : PE matmul qlT-slice x pos_kT                    [112, 17]
      p2c band   : PE matmul klT-slice x pos_qT (per key tile)     [128, 17]
  - Band->block diagonal shear is done through small DRAM bounce buffers
    written contiguously ([n,17] row-major) and read back with a sheared
    access pattern (row step 16, inner step 1) -> fully contiguous bursts.
  - p2c is folded in multiplicatively after exp (E2 = exp(p2c/scale), stored
    pre-exp'd in its bounce buffer; zero padding doubles as the key-validity
    mask).  P~^T = transpose(exp(c2c+c2p+mask)) * E2 feeds the PE directly:
      ctx/Z matmul: lhsT = P~^T [win, bsz], rhs = [vl | ones]      [bsz, 128]
  - ctx head slice = ctx / Z, + bv broadcast, streamed out.
"""
import sys
import numpy as np

if '/opt/trn_rl_repo' not in sys.path:
    sys.path.insert(0, '/opt/trn_rl_repo')

import concourse.bass as bass
import concourse.bacc as bacc
import concourse.tile as tile
from concourse import mybir
from concourse.masks import make_identity
from contextlib import ExitStack

F32 = mybir.dt.float32
BF16 = mybir.dt.bfloat16
F32R = mybir.dt.float32r

B, N, D, H = 16, 512, 512, 8
d = D // H                      # 64
S, W = 512, 8
SCALE = float(np.sqrt(d * 3))   # sqrt(192)
NCORES = 8
BB = B // NCORES                # batches per core = 2

QB = [0, 112, 224, 336, 448]    # query block starts
QS = [112, 112, 112, 112, 64]   # query block sizes
K0 = [0, 104, 216, 328, 440]    # key window starts (clipped to [0, N))
WIN = [128, 128, 128, 128, 72]  # key window sizes (clipped to N)
NT = D // 128                   # 4 c-tiles
PITCH = 17
NPAD = 576                      # rows in each band bounce buffer
KPAD = 576                      # vl halo tile free size (512 data + 64 ones)

# matmul input dtype for the projection stage.  Plain f32 matmuls are avoided
# everywhere: their hi/lo two-pass codegen only supports one sync wait.
PROJ_DT = BF16
ATTN_DT = BF16


def _build_program():
    nc = bacc.Bacc(None, target_bir_lowering=False)
    q_sh = nc.declare_dram_parameter("q_sh", [BB * N, D], F32, isOutput=False)
    k_sh = nc.declare_dram_parameter("k_sh", [BB * N, D], F32, isOutput=False)
    v_sh = nc.declare_dram_parameter("v_sh", [BB * N, D], F32, isOutput=False)
    Wq = nc.declare_dram_parameter("Wq", [D, D], F32, isOutput=False)
    Wk = nc.declare_dram_parameter("Wk", [D, D], F32, isOutput=False)
    Wv = nc.declare_dram_parameter("Wv", [D, D], F32, isOutput=False)
    bq = nc.declare_dram_parameter("bq", [D], F32, isOutput=False)
    bk = nc.declare_dram_parameter("bk", [D], F32, isOutput=False)
    bv = nc.declare_dram_parameter("bv", [D], F32, isOutput=False)
    relb = nc.declare_dram_parameter("relb", [17, D], F32, isOutput=False)
    o_sh = nc.declare_dram_parameter("o_sh", [BB * N, D], F32, isOutput=True)

    # band bounce buffers, one row-region per (bb, h)
    cpb = nc.dram_tensor("cpb", [BB * H * NPAD * PITCH], BF16)
    kpb = nc.dram_tensor("kpb", [BB * H * NPAD * PITCH], BF16)

    with tile.TileContext(nc) as tc, ExitStack() as ctx:
        cpool = ctx.enter_context(tc.tile_pool(name="const", bufs=1))
        stage = ctx.enter_context(tc.tile_pool(name="stage", bufs=4))
        xtp = ctx.enter_context(tc.tile_pool(name="xt", bufs=1))
        projp = ctx.enter_context(tc.tile_pool(name="proj", bufs=1))
        workp = ctx.enter_context(tc.tile_pool(name="work", bufs=2))
        psum = ctx.enter_context(tc.tile_pool(name="psum", bufs=2, space="PSUM"))
        psmall = ctx.enter_context(tc.tile_pool(name="psmall", bufs=2, space="PSUM"))

        # ---------------- setup: constants -------------------------------
        identB = cpool.tile([128, 128], BF16, tag="identB")
        make_identity(nc, identB[:])

        # band masks {0,1}: interior blocks keep 0 <= m_l-n_l <= 16,
        # block 0 keeps -8 <= m_l-n_l <= 8 (its key window starts at key 0)
        ones = cpool.tile([112, 128], BF16, tag="ones_m")
        nc.vector.memset(ones[:], 1.0)

        def make_band_mask(lo, hi, tag):
            t0 = cpool.tile([112, 128], BF16, tag=tag + "_a", name=tag + "_a")
            nc.gpsimd.affine_select(t0[:], ones[:], [[1, 128]],
                                    mybir.AluOpType.is_ge, 0.0,
                                    base=-lo, channel_multiplier=-1)
            t1 = cpool.tile([112, 128], BF16, tag=tag, name=tag)
            nc.gpsimd.affine_select(t1[:], t0[:], [[-1, 128]],
                                    mybir.AluOpType.is_ge, 0.0,
                                    base=hi, channel_multiplier=1)
            return t1

        bmaskA = make_band_mask(0, 16, "bmaskA")   # interior
        bmaskB = make_band_mask(-8, 8, "bmaskB")   # first block

        # zero-init the bounce buffers (pads must be finite; kp pads must be 0)
        zsb = cpool.tile([128, BB * H * NPAD * PITCH // 128], BF16, tag="zsb")
        nc.vector.memset(zsb[:], 0.0)
        zap = bass.AP(cpb, 0, [[zsb.shape[1], 128], [1, zsb.shape[1]]])
        nc.sync.dma_start(zap, zsb[:])
        zap2 = bass.AP(kpb, 0, [[zsb.shape[1], 128], [1, zsb.shape[1]]])
        nc.sync.dma_start(zap2, zsb[:])

        # biases: bqTs[i] = bq/scale, bkT[i] = bk, per-partition columns
        bqTs, bkT = [], []
        for i in range(NT):
            t = cpool.tile([128, 1], F32, tag=f"bq{i}")
            nc.sync.dma_start(t[:], bq[128 * i:128 * (i + 1)])
            ts = cpool.tile([128, 1], F32, tag=f"bqs{i}")
            nc.scalar.mul(ts[:], t[:], 1.0 / SCALE)
            bqTs.append(ts)
            tk = cpool.tile([128, 1], F32, tag=f"bk{i}")
            nc.sync.dma_start(tk[:], bk[128 * i:128 * (i + 1)])
            bkT.append(tk)
        bvb = cpool.tile([128, D], F32, tag="bvb")
        nc.sync.dma_start(bvb[:], bass.AP(bv, 0, [[0, 128], [1, D]]))

        # prime the DVE vector clock on constant tiles so their later
        # consumers (1-wait-limited TensorScalarPtr ops etc.) need no DMA wait
        for _i, _t in enumerate(bqTs + bkT + [bvb]):
            _p = cpool.tile([128, 1], F32, tag=f"prime{_i}", name=f"prime{_i}")
            nc.vector.tensor_add(_p[:], _t[0:128, 0:1], _t[0:128, 0:1])
        for _i, _t in enumerate([bmaskA, bmaskB]):
            _p = cpool.tile([112, 1], BF16, tag=f"primb{_i}", name=f"primb{_i}")
            nc.vector.tensor_add(_p[:], _t[0:112, 0:1], _t[0:112, 0:1])

        # ---------------- setup: weight transposes -----------------------
        def load_transposed(W_dram, out_dt, tag):
            """Return 4 tiles [128(c), 512(c_out)] = W.T in out_dt."""
            wn = []
            for i in range(NT):
                t = stage.tile([128, D], out_dt, tag=f"ws_{tag}{i}",
                               name=f"ws_{tag}{i}", bufs=1)
                nc.gpsimd.dma_start(t[:], W_dram[128 * i:128 * (i + 1), :])
                wn.append(t)
            wt = [cpool.tile([128, D], out_dt, tag=f"{tag}{j}", name=f"{tag}{j}")
                  for j in range(NT)]
            for i in range(NT):          # c_out tile of natural W
                for j in range(NT):      # c tile
                    pt = psum.tile([128, 128], out_dt, tag="tp")
                    nc.tensor.transpose(pt[:], wn[i][:, 128 * j:128 * (j + 1)],
                                        identB[:])
                    nc.scalar.copy(wt[j][:, 128 * i:128 * (i + 1)], pt[:])
            return wt

        WqT = load_transposed(Wq, PROJ_DT, "wqt")
        WkT = load_transposed(Wk, PROJ_DT, "wkt")
        WvT = load_transposed(Wv, PROJ_DT, "wvt")

        # ---------------- setup: positional projections ------------------
        # relT[j] [128, 17] = relb.T slice (c rows 128j..), ascending pos rows
        rel_s = stage.tile([17, D], PROJ_DT, tag="rels")
        nc.gpsimd.dma_start(rel_s[:], relb[:])
        relT = []
        for j in range(NT):
            pt = psum.tile([128, 17], PROJ_DT, tag="tp")
            nc.tensor.transpose(pt[:], rel_s[:, 128 * j:128 * (j + 1)],
                                identB[0:17, 0:17])
            rt = cpool.tile([128, 17], PROJ_DT, tag=f"relT{j}")
            nc.scalar.copy(rt[:], pt[:])
            relT.append(rt)

        # pos_kT[o][co, t] = (relb @ Wk.T + bk).T    (t ascending = pos 504+t)
        # pos_qT[o] = ((relb @ Wq.T + bq)/scale).T
        pkT, pqT = [], []
        for o in range(NT):
            pk = psum.tile([128, 17], F32, tag="tp")
            for i in range(NT):
                nc.tensor.matmul(pk[:], WkT[i][:, 128 * o:128 * (o + 1)],
                                 relT[i][:], start=(i == 0), stop=(i == NT - 1))
            t = cpool.tile([128, 17], ATTN_DT, tag=f"pkT{o}")
            nc.scalar.copy(t[:], pk[:])
            nc.vector.tensor_scalar_add(t[:], t[:], bkT[o][:])
            pkT.append(t)
            pq = psum.tile([128, 17], F32, tag="tp")
            for i in range(NT):
                nc.tensor.matmul(pq[:], WqT[i][:, 128 * o:128 * (o + 1)],
                                 relT[i][:], start=(i == 0), stop=(i == NT - 1))
            t2 = cpool.tile([128, 17], ATTN_DT, tag=f"pqT{o}")
            nc.scalar.mul(t2[:], pq[:], 1.0 / SCALE)
            nc.vector.tensor_scalar_add(t2[:], t2[:], bqTs[o][:])
            pqT.append(t2)

        # ---------------- per-batch pipeline ------------------------------
        for bb in range(BB):
            row0 = bb * N

            # -- load + transpose inputs:  xT[i] = x.T slice [c rows, n]
            def load_T(x_dram, tag):
                xs = []
                for jb in range(NT):
                    t = stage.tile([128, D], PROJ_DT, tag=f"xs_{tag}{jb}",
                                   name=f"xs_{tag}{jb}", bufs=1)
                    nc.gpsimd.dma_start(t[:], x_dram[row0 + 128 * jb:
                                                     row0 + 128 * (jb + 1), :])
                    xs.append(t)
                xt = [xtp.tile([128, D], PROJ_DT, tag=f"{tag}{i}",
                               name=f"{tag}{i}") for i in range(NT)]
                for jb in range(NT):
                    for i in range(NT):
                        pt = psum.tile([128, 128], PROJ_DT, tag="tp")
                        nc.tensor.transpose(pt[:], xs[jb][:, 128 * i:128 * (i + 1)],
                                            identB[:])
                        nc.scalar.copy(xt[i][:, 128 * jb:128 * (jb + 1)], pt[:])
                return xt

            qT = load_T(q_sh, f"qT{bb}_")
            kT = load_T(k_sh, f"kT{bb}_")
            vT = load_T(v_sh, f"vT{bb}_")

            # -- projections
            qlT, klT = [], []
            for o in range(NT):
                pj = psum.tile([128, D], F32, tag="big")
                for i in range(NT):
                    nc.tensor.matmul(pj[:], WqT[i][:, 128 * o:128 * (o + 1)],
                                     qT[i][:], start=(i == 0), stop=(i == NT - 1))
                t = projp.tile([128, D], ATTN_DT, tag=f"qlT{bb}_{o}")
                nc.scalar.mul(t[:], pj[:], 1.0 / SCALE)
                nc.vector.tensor_scalar_add(t[:], t[:], bqTs[o][:])
                qlT.append(t)

                pj2 = psum.tile([128, D], F32, tag="big")
                for i in range(NT):
                    nc.tensor.matmul(pj2[:], WkT[i][:, 128 * o:128 * (o + 1)],
                                     kT[i][:], start=(i == 0), stop=(i == NT - 1))
                tk = projp.tile([128, N], ATTN_DT, tag=f"klT{bb}_{o}")
                nc.scalar.copy(tk[:], pj2[:])
                nc.vector.tensor_scalar_add(tk[:], tk[:], bkT[o][:])
                klT.append(tk)

            # vl halo tiles: rows = keys K0[t] .. K0[t]+WIN[t]
            vlh = []
            for t5 in range(5):
                k0, cnt = K0[t5], WIN[t5]
                pj = psum.tile([128, D], F32, tag="big")
                for i in range(NT):
                    nc.tensor.matmul(pj[0:cnt, :], vT[i][:, k0:k0 + cnt],
                                     WvT[i][:], start=(i == 0), stop=(i == NT - 1))
                tv = projp.tile([128, KPAD], ATTN_DT, tag=f"vlh{bb}_{t5}")
                nc.vector.memset(tv[:, D:KPAD], 1.0)
                nc.scalar.copy(tv[0:cnt, 0:D], pj[0:cnt, :])
                vlh.append(tv)

            # per-(bb, qb) output tiles, filled one head-slice at a time
            osbs = [projp.tile([112, D], F32, tag=f"osb{bb}_{i}",
                               name=f"osb{bb}_{i}") for i in range(5)]

            # -- attention per head
            for h in range(H):
                ot, po = h // 2, (h % 2) * 64
                bh = bb * H + h
                kp_base = bh * NPAD * PITCH
                cp_base = bh * NPAD * PITCH

                # p2c -> E2, written pre-exp'd to kpb
                for mt in range(NT):
                    pk = psum.tile([128, 17], F32, tag="big")
                    nc.tensor.matmul(pk[:],
                                     klT[ot][po:po + 64,
                                             128 * mt:128 * (mt + 1)],
                                     pqT[ot][po:po + 64, :],
                                     start=True, stop=True)
                    e2s = workp.tile([128, 17], BF16, tag="e2s")
                    nc.scalar.activation(e2s[:], pk[:],
                                         mybir.ActivationFunctionType.Exp)
                    wap = bass.AP(kpb, kp_base + (8 + 128 * mt) * PITCH,
                                  [[PITCH, 128], [1, 17]])
                    nc.sync.dma_start(wap, e2s[:])

                # query blocks
                for qb in range(5):
                    q0, bsz, win, k0 = QB[qb], QS[qb], WIN[qb], K0[qb]
                    bmask = bmaskB if qb == 0 else bmaskA

                    sc = psum.tile([128, 160], F32, tag="big")
                    nc.tensor.matmul(sc[0:bsz, 0:win],
                                     qlT[ot][po:po + 64, q0:q0 + bsz],
                                     klT[ot][po:po + 64, k0:k0 + win],
                                     start=True, stop=False)
                    nc.tensor.matmul(sc[0:bsz, 128:145],
                                     qlT[ot][po:po + 64, q0:q0 + bsz],
                                     pkT[ot][po:po + 64, :],
                                     start=False, stop=True)

                    # c2p band -> DRAM (reversed cols: t = 16 - j)
                    cps = workp.tile([128, 17], BF16, tag="cps")
                    nc.scalar.copy(cps[0:bsz, :], sc[0:bsz, 128:145])
                    wap = bass.AP(cpb, cp_base + (8 + q0) * PITCH + 16,
                                  [[PITCH, bsz], [-1, 17]])
                    nc.sync.dma_start(wap, cps[0:bsz, :])

                    # sheared reads (blk element (n_l, m_l) -> band buffers)
                    cpk = workp.tile([112, 128], BF16, tag="cpk")
                    rap = bass.AP(cpb, cp_base + (8 + q0) * PITCH + k0 - q0 + 8,
                                  [[16, bsz], [1, win]])
                    nc.sync.dma_start(cpk[0:bsz, 0:win], rap)
                    e2b = workp.tile([128, 112], BF16, tag="e2b")
                    rap2 = bass.AP(kpb, kp_base + (8 + k0) * PITCH + q0 - k0 + 8,
                                   [[16, win], [1, bsz]])
                    nc.sync.dma_start(e2b[0:win, 0:bsz], rap2)

                    # E1 = exp(c2c + c2p) * bandmask
                    t1 = workp.tile([112, 128], F32, tag="t1")
                    nc.vector.tensor_add(t1[0:bsz, 0:win], sc[0:bsz, 0:win],
                                         cpk[0:bsz, 0:win])
                    e1r = workp.tile([112, 128], BF16, tag="e1r")
                    nc.scalar.activation(e1r[0:bsz, 0:win], t1[0:bsz, 0:win],
                                         mybir.ActivationFunctionType.Exp)
                    e1 = workp.tile([112, 128], BF16, tag="e1")
                    nc.vector.tensor_mul(e1[0:bsz, 0:win], e1r[0:bsz, 0:win],
                                         bmask[0:bsz, 0:win])

                    # P~^T = E1^T * E2
                    ptp = psmall.tile([128, 112], BF16, tag="ptp")
                    nc.tensor.transpose(ptp[0:win, 0:bsz], e1[0:bsz, 0:win],
                                        identB[0:bsz, 0:bsz])
                    ptt = workp.tile([128, 112], BF16, tag="ptt")
                    nc.vector.tensor_mul(ptt[0:win, 0:bsz], ptp[0:win, 0:bsz],
                                         e2b[0:win, 0:bsz])

                    # ctx + Z in one matmul: rhs = [vl_h | ones]
                    cz = psmall.tile([112, 128], F32, tag="cz")
                    vt = vlh[qb][:]
                    rhs = bass.AP(vt.tensor, vt.offset + h * 64,
                                  [[vt.ap[0][0], win], [D - h * 64, 2], [1, 64]])
                    nc.tensor.matmul(cz[0:bsz, :], ptt[0:win, 0:bsz], rhs,
                                     start=True, stop=True)

                    rz = workp.tile([112, 1], F32, tag="rz")
                    nc.vector.reciprocal(rz[0:bsz, :], cz[0:bsz, 64:65])
                    osb = osbs[qb]
                    nc.vector.tensor_scalar_mul(osb[0:bsz, h * 64:h * 64 + 64],
                                                cz[0:bsz, 0:64], rz[0:bsz, :])
                    if h == H - 1:
                        nc.vector.tensor_add(osb[0:bsz, :], osb[0:bsz, :],
                                             bvb[0:bsz, :])
                        nc.sync.dma_start(
                            o_sh[row0 + q0:row0 + q0 + bsz, :], osb[0:bsz, :])
    nc.finalize()
    return nc


_CACHE = {}


def _get_program():
    if "nc" not in _CACHE:
        _CACHE["nc"] = _build_program()
    return _CACHE["nc"]


def kernel(q, k, v, rel_emb, Wq, bq, Wk, bk, Wv, bv):
    q = np.ascontiguousarray(q, np.float32)
    k = np.ascontiguousarray(k, np.float32)
    v = np.ascontiguousarray(v, np.float32)
    relb = np.ascontiguousarray(rel_emb[S - W:S + W + 1], np.float32)  # 17 rows
    common = {
        "Wq": np.ascontiguousarray(Wq, np.float32),
        "Wk": np.ascontiguousarray(Wk, np.float32),
        "Wv": np.ascontiguousarray(Wv, np.float32),
        "bq": np.ascontiguousarray(bq, np.float32),
        "bk": np.ascontiguousarray(bk, np.float32),
        "bv": np.ascontiguousarray(bv, np.float32),
        "relb": relb,
    }
    in_maps = []
    for c in range(NCORES):
        sl = slice(BB * c, BB * (c + 1))
        in_maps.append({
            "q_sh": q[sl].reshape(BB * N, D),
            "k_sh": k[sl].reshape(BB * N, D),
            "v_sh": v[sl].reshape(BB * N, D),
            **common,
        })

    from concourse.bass_utils import run_bass_kernel_spmd
    nc = _get_program()
    res = run_bass_kernel_spmd(nc, in_maps, list(range(NCORES)))
    out = np.empty((B, N, D), np.float32)
    for c in range(NCORES):
        out[BB * c:BB * (c + 1)] = res.results[c]["o_sh"].reshape(BB, N, D)
    return out


if __name__ == "__main__":
    # quick CoreSim smoke test of one core against a local numpy reference
    from concourse.bass_interp import CoreSim
    rng = np.random.default_rng(0)
    nc = _build_program()
    print("program built")
